# revision 20
# baseline (speedup 1.0000x reference)
"""Trainium2 Bass kernel: causal multi-head self-attention with RoPE.

Problem: B=4, S=2048, D=1024, H=16, DK=64.  out = softmax(causal(qk^T/8)) v @ wo^T
with q,k RoPE-rotated.

Sharding: 8 cores = (batch b in 0..3) x (head-group g in 0..1, 8 heads each).
Each core computes its batch's QKV for its 8 heads, causal attention, and a
partial output projection; the host sums the two group-partials per batch.

Precision plan (validated on host): the softmax averages ~0.78*r keys at query
row r, so fp8e4m3 noise in E/V/Q/K washes out for r >= 512 but not below.
Everything touching query rows < 512 (tn=0 projections, qc=0 attention) and
keys < 512 for the qc=0 block stays bf16; the value path for qc>=1 runs fp8
with perf_mode=DoubleRow (2 k-subtiles per matmul).  The output projection
stays bf16 (fp8 relative error there hits the final result directly).

PE model (measured): matmul wall time ~ output free size x ~0.73-1.23ns
(drain-bound, clock depends on the HAM state); DoubleRow wins only by folding
2 contraction chunks into one output drain.  Scores are emitted as 64x64
quadrant-tiled matmuls (tile_position row+col groups) so up to 4 drain
concurrently, and the diagonal band skips fully-masked column prefixes
(stale PSUM there is bounded and the causal mask zeroes it; the st pool is
memset once at start).

Schedule: flattened (pair, qc) blocks with each pair's small qc=0 block
interleaved into the previous pair's big attention stream, erasing the
pair-boundary dips; next-pair q/k projections and the last pair's output
projections drip into the attention stream as filler.  Output partials are
written bf16 and summed on host.
"""
import os
import sys

for _p in ("/opt/trn_rl_repo", "/root/.axon_site/_ro/trn_rl_repo"):
    if os.path.isdir(_p) and _p not in sys.path:
        sys.path.insert(0, _p)

import numpy as np
import ml_dtypes

import concourse.bass as bass
import concourse.mybir as mybir
import concourse.tile as tile
from concourse import bacc
from concourse.bass_utils import run_bass_kernel_spmd

B, S, D, H = 4, 2048, 1024, 16
DK = D // H          # 64
HG = 8               # heads per group
NG = 2               # head groups (cores per batch)
THETA = 10000.0
NCORES = 8

BF16 = mybir.dt.bfloat16
F8 = mybir.dt.float8e4
F32 = mybir.dt.float32
bf16 = ml_dtypes.bfloat16
f8e4 = ml_dtypes.float8_e4m3

QT = 512             # q tile width (free dim)
NQT = S // QT        # 4
NKT = S // 128       # 16 k chunks
NJT = HG * DK // 128  # 4 j-tiles (head pairs)
NDC = D // 128       # 8 d chunks
NMT = D // 128       # 8 output m tiles

DR = mybir.MatmulPerfMode.DoubleRow


def _build_nc():
    nc = bacc.Bacc("TRN2", target_bir_lowering=False, debug=False)
    xTb = nc.dram_tensor("xTb", [D, QT], BF16, kind="ExternalInput").ap()
    x8T = nc.dram_tensor("x8T", [D, S], F8, kind="ExternalInput").ap()
    wqT = nc.dram_tensor("wqT", [D, HG * DK], BF16, kind="ExternalInput").ap()
    wkT = nc.dram_tensor("wkT", [D, HG * DK], BF16, kind="ExternalInput").ap()
    wq8T = nc.dram_tensor("wq8T", [D, HG * DK], F8, kind="ExternalInput").ap()
    wk8T = nc.dram_tensor("wk8T", [D, HG * DK], F8, kind="ExternalInput").ap()
    wvT = nc.dram_tensor("wvT", [D, HG * DK], BF16, kind="ExternalInput").ap()
    wv8T = nc.dram_tensor("wv8T", [D, HG * DK], F8, kind="ExternalInput").ap()
    woT = nc.dram_tensor("woT", [HG * DK, D], BF16, kind="ExternalInput").ap()
    c128 = nc.dram_tensor("c128", [128, S], BF16, kind="ExternalInput").ap()
    s128 = nc.dram_tensor("s128", [128, S], BF16, kind="ExternalInput").ap()
    maskd = nc.dram_tensor("maskd", [128, 4, QT], BF16, kind="ExternalInput").ap()
    outT = nc.dram_tensor("outT", [D, S], BF16, kind="ExternalOutput").ap()

    from contextlib import ExitStack
    with tile.TileContext(nc) as tc, ExitStack() as stk:
        pp = stk.enter_context(tc.tile_pool(name="persist", bufs=1))
        ep = stk.enter_context(tc.tile_pool(name="epool", bufs=6))
        sp = stk.enter_context(tc.tile_pool(name="smalls", bufs=2))
        qw = stk.enter_context(tc.tile_pool(name="qkvwork", bufs=2))
        ps_st = stk.enter_context(
            tc.tile_pool(name="ps_st", bufs=2, space="PSUM"))
        ps_ov = stk.enter_context(
            tc.tile_pool(name="ps_ov", bufs=2, space="PSUM"))
        ps_qkv = stk.enter_context(
            tc.tile_pool(name="ps_qkv", bufs=2, space="PSUM"))

        # ---------------- persistent tiles ----------------
        wo_sb = pp.tile([128, NJT, D], BF16)
        m_sb = pp.tile([128, 4, QT], BF16)
        qrot = pp.tile([128, NJT, S], BF16)
        krot = pp.tile([128, NJT, S], BF16)
        v_aug = pp.tile([128, 4, HG, 66], BF16)     # bf16 kc<4 (qc=0 path)
        v8 = pp.tile([128, NKT, HG, 66], F8)        # fp8 all kc (qc>=1 path)
        a_t = pp.tile([128, NJT, S], BF16)
        xb_sb = pp.tile([128, NDC, QT], BF16)       # x cols 0..511 (bf16 path)
        x8_sb = pp.tile([128, NDC, S - QT], F8)     # x cols 512.. (fp8 paths)
        wq_sb = pp.tile([128, NDC, HG * DK], BF16)
        wk_sb = pp.tile([128, NDC, HG * DK], BF16)
        wq8_sb = pp.tile([128, NDC, HG * DK], F8)
        wk8_sb = pp.tile([128, NDC, HG * DK], F8)
        c_sb = pp.tile([128, S], BF16)
        s_sb = pp.tile([128, S], BF16)

        nc.gpsimd.memset(v_aug[:, :, :, 64:65], 1.0)
        nc.gpsimd.memset(v8[:, :, :, 64:65], 1.0)
        # the diagonal-band score matmuls skip fully-masked column prefixes,
        # so exp() can read stale PSUM there (the mask zeroes it afterwards);
        # zero the two st pool buffers once so "stale" is always bounded.
        for i in range(2):
            sti = ps_st.tile([128, 2 * QT], F32, tag="st", name=f"stini{i}")
            nc.vector.memset(sti[:], 0.0)

        # ---------------- v projection (all heads) ----------------
        # bf16 for the first 4 k-chunks (they feed the bf16 qc=0 path),
        # fp8 DoubleRow for the rest; everything lands in v8, the bf16
        # chunks additionally in v_aug.
        with tc.tile_pool(name="wvtmp", bufs=1) as wvp:
            wv_sb = wvp.tile([128, NDC, HG * DK], BF16)
            wv8_sb = wvp.tile([128, NDC, HG * DK], F8)
            # minimal prologue: the first bf16 v-proj matmul needs only
            # wv + xb; everything else streams in behind it on 2 queues.
            for dc in range(NDC):
                nc.sync.dma_start(wv_sb[:, dc, :],
                                  wvT[dc * 128:(dc + 1) * 128, :])
                nc.sync.dma_start(xb_sb[:, dc, :],
                                  xTb[dc * 128:(dc + 1) * 128, :])
            nc.gpsimd.dma_start(
                x8_sb[:],
                x8T[:, QT:].rearrange("(dc p) c -> p dc c", p=128))
            nc.gpsimd.dma_start(
                wv8_sb[:],
                wv8T[:].rearrange("(dc p) c -> p dc c", p=128))
            nc.sync.dma_start(
                wq_sb[:], wqT[:].rearrange("(dc p) c -> p dc c", p=128))
            nc.sync.dma_start(
                wk_sb[:], wkT[:].rearrange("(dc p) c -> p dc c", p=128))
            nc.gpsimd.dma_start(
                wq8_sb[:], wq8T[:].rearrange("(dc p) c -> p dc c", p=128))
            nc.gpsimd.dma_start(
                wk8_sb[:], wk8T[:].rearrange("(dc p) c -> p dc c", p=128))
            nc.sync.dma_start(
                wo_sb[:], woT[:].rearrange("(jc p) c -> p jc c", p=128))
            nc.sync.dma_start(c_sb[:], c128[:])
            nc.sync.dma_start(s_sb[:], s128[:])
            nc.sync.dma_start(m_sb[:], maskd[:])
            for tt in range(NKT):
                ps = ps_qkv.tile([128, QT], F32, tag="qv")
                if tt < 4:
                    for dc in range(NDC):
                        nc.tensor.matmul(
                            ps[:],
                            xb_sb[:, dc, tt * 128:(tt + 1) * 128],
                            wv_sb[:, dc, :],
                            start=(dc == 0), stop=(dc == NDC - 1))
                    nc.scalar.copy(
                        v_aug[:, tt, :, 0:64],
                        ps[:].rearrange("p (h d) -> p h d", h=HG))
                else:
                    for dc2 in range(NDC // 2):
                        nc.tensor.matmul(
                            ps[:],
                            x8_sb[:, 2 * dc2:2 * dc2 + 2,
                                  tt * 128 - QT:(tt + 1) * 128 - QT],
                            wv8_sb[:, 2 * dc2:2 * dc2 + 2, :],
                            start=(dc2 == 0), stop=(dc2 == NDC // 2 - 1),
                            perf_mode=DR)
                nc.vector.tensor_copy(
                    v8[:, tt, :, 0:64],
                    ps[:].rearrange("p (h d) -> p h d", h=HG))

        # ------------- projections + interleaved attention ---------
        def proj_unit(pair, name, w_sb, w8_sb, pre, tn):
            ps = ps_qkv.tile([128, QT], F32, tag="qv",
                             name=f"ps{name}{pair}{tn}")
            if tn == 0:
                for dc in range(NDC):
                    nc.tensor.matmul(
                        ps[:],
                        w_sb[:, dc, pair * 128:(pair + 1) * 128],
                        xb_sb[:, dc, :],
                        start=(dc == 0), stop=(dc == NDC - 1))
            else:
                for dc2 in range(NDC // 2):
                    nc.tensor.matmul(
                        ps[:],
                        w8_sb[:, 2 * dc2:2 * dc2 + 2,
                              pair * 128:(pair + 1) * 128],
                        x8_sb[:, 2 * dc2:2 * dc2 + 2,
                              (tn - 1) * QT:tn * QT],
                        start=(dc2 == 0), stop=(dc2 == NDC // 2 - 1),
                        perf_mode=DR)
            nc.vector.tensor_copy(pre[:, tn * QT:(tn + 1) * QT], ps[:])

        def rope_unit(pair, name, pre, dst):
            swp = qw.tile([128, S], BF16, tag="swp", name=f"swp{name}{pair}")
            for a in range(4):
                lo, sw = 32 * a, 32 * (a ^ 1)
                q = nc.sync if a % 2 == 0 else nc.gpsimd
                q.dma_start(swp[lo:lo + 32, :], pre[sw:sw + 32, :])
            nc.vector.tensor_mul(dst[:, pair, :], pre[:], c_sb[:])
            nc.vector.tensor_mul(swp[:], swp[:], s_sb[:])
            nc.vector.tensor_add(dst[:, pair, :], dst[:, pair, :], swp[:])

        def emit_scores(pair, qc, g):
            st0 = ps_st.tile([128, 2 * QT], F32, tag="st",
                             name=f"st0_{pair}{qc}{g}")
            st1 = ps_st.tile([128, 2 * QT], F32, tag="st",
                             name=f"st1_{pair}{qc}{g}")
            diag = g >= 2 * qc
            for half in range(2):
                kc = 2 * g + half
                for h01, st in ((0, st0), (1, st1)):
                    for kh in range(2):
                        qlo = 0
                        if diag:
                            jloc = kc - 4 * qc   # 0..3 within the band
                            qlo = 128 * jloc + 64 * kh
                        nc.tensor.matmul(
                            st[64 * kh:64 * kh + 64,
                               half * QT + qlo:(half + 1) * QT],
                            krot[64 * h01:64 * h01 + 64, pair,
                                 kc * 128 + 64 * kh:kc * 128 + 64 * kh + 64],
                            qrot[64 * h01:64 * h01 + 64, pair,
                                 qc * QT + qlo:(qc + 1) * QT],
                            start=True, stop=True,
                            tile_position=(64 * h01, 64 * kh))
            return st0, st1

        def emit_tail(pair, qc, g, st0, st1, ov0, ov1, last):
            diag = g >= 2 * qc
            for h01, st, ov in ((0, st0, ov0), (1, st1, ov1)):
                # exp lands in bf16 whenever a mask multiply follows (the
                # 16-bit DVE path is 2x the fp8 one); the mask multiply then
                # converts to fp8 on the way out for the DoubleRow matmul.
                edt = BF16 if (qc == 0 or diag) else F8
                e = ep.tile([128, 2 * QT], edt, tag="e", bufs=5,
                            name=f"e{pair}{qc}{g}{h01}")
                nc.scalar.activation(
                    e[:], st[:], mybir.ActivationFunctionType.Exp,
                    scale=0.125)
                if diag:
                    par = g - 2 * qc
                    if qc == 0:
                        e3 = e[:].rearrange("p (a q) -> p a q", a=2)
                        nc.vector.tensor_mul(
                            e3, e3, m_sb[:, 2 * par:2 * par + 2, :])
                    else:
                        e8 = ep.tile([128, 2 * QT], F8, tag="e8",
                                     bufs=3, name=f"e8{pair}{qc}{g}{h01}")
                        nc.gpsimd.tensor_mul(
                            e8[:].rearrange("p (a q) -> p a q", a=2),
                            e[:].rearrange("p (a q) -> p a q", a=2),
                            m_sb[:, 2 * par:2 * par + 2, :])
                        e = e8
                if qc == 0:
                    for half in range(2):
                        kc = 2 * g + half
                        nc.tensor.matmul(
                            ov[:],
                            v_aug[:, kc, 2 * pair + h01, 0:65],
                            e[:, half * QT:(half + 1) * QT],
                            start=(kc == 0),
                            stop=(last and half == 1))
                else:
                    nc.tensor.matmul(
                        ov[:],
                        v8[:, 2 * g:2 * g + 2, 2 * pair + h01, 0:65],
                        e[:].rearrange("p (two q) -> p two q", two=2),
                        start=(g == 0),
                        stop=last,
                        perf_mode=DR)

        def emit_evac(pair, qc, ov0, ov1):
            """Part A: free the ov PSUM banks and stage the denominators."""
            den = sp.tile([2, QT], F32, tag="den", bufs=2,
                          name=f"den{pair}{qc}")
            ous = []
            for h01, ov in ((0, ov0), (1, ov1)):
                ou = ep.tile([65, QT], BF16, tag="ou", bufs=5,
                             name=f"ou{pair}{qc}{h01}")
                nc.vector.tensor_copy(ou[:], ov[:])
                nc.gpsimd.dma_start(den[h01:h01 + 1, :], ou[64:65, :])
                ous.append(ou)
            return den, ous

        def emit_recip(pair, qc, den):
            """Part B1: reciprocal + broadcast DMAs (deferred one qc)."""
            recip = sp.tile([2, QT], F32, tag="recip", name=f"rcp{pair}{qc}")
            nc.vector.reciprocal_approx_fast(recip[:], den[:])
            rbs = []
            for h01 in range(2):
                rb = sp.tile([64, QT], BF16, tag="rb", bufs=4,
                             name=f"rb{pair}{qc}{h01}")
                nc.gpsimd.dma_start(
                    rb[:],
                    recip[h01:h01 + 1, :]
                    .unsqueeze(1).to_broadcast((1, 64, QT)))
                rbs.append(rb)
            return rbs

        def emit_div(pair, qc, ous, rbs):
            """Part B2: the normalize multiplies (deferred further)."""
            nc.vector.tensor_mul(
                a_t[0:64, pair, qc * QT:(qc + 1) * QT],
                ous[0][0:64, :], rbs[0][:])
            an = sp.tile([64, QT], BF16, tag="an", bufs=3,
                         name=f"an{pair}{qc}")
            nc.vector.tensor_mul(an[:], ous[1][0:64, :], rbs[1][:])
            nc.sync.dma_start(
                a_t[64:128, pair, qc * QT:(qc + 1) * QT], an[:])

        def outproj_unit(qc, mt):
            op = ps_qkv.tile([128, QT], F32, tag="qv", name=f"op{qc}{mt}")
            for jc in range(NJT):
                nc.tensor.matmul(
                    op[:],
                    wo_sb[:, jc, mt * 128:(mt + 1) * 128],
                    a_t[:, jc, qc * QT:(qc + 1) * QT],
                    start=(jc == 0), stop=(jc == NJT - 1))
            ot = sp.tile([128, QT], BF16, tag="ot", bufs=3,
                         name=f"ot{qc}{mt}")
            nc.vector.tensor_copy(ot[:], op[:])
            nc.sync.dma_start(
                outT[mt * 128:(mt + 1) * 128, qc * QT:(qc + 1) * QT],
                ot[:])

        def proj_units(pair):
            preq = qw.tile([128, S], BF16, tag="preq", name=f"preq{pair}")
            prek = qw.tile([128, S], BF16, tag="prek", name=f"prek{pair}")
            for tn in range(NQT):
                yield lambda tn=tn: proj_unit(pair, "q", wq_sb, wq8_sb,
                                              preq, tn)
            yield lambda: rope_unit(pair, "q", preq, qrot)
            for tn in range(NQT):
                yield lambda tn=tn: proj_unit(pair, "k", wk_sb, wk8_sb,
                                              prek, tn)
            yield lambda: rope_unit(pair, "k", prek, krot)

        from collections import deque
        filler = deque()

        # flattened block order: each pair's small qc=0 block runs inside the
        # previous pair's attention stream, erasing the pair-boundary dip
        blocks = [(0, 0), (0, 1), (0, 2), (1, 0), (0, 3), (1, 1), (1, 2),
                  (2, 0), (1, 3), (2, 1), (2, 2), (3, 0), (2, 3), (3, 1),
                  (3, 2), (3, 3)]

        def run_block(pair, qc):
            ngrp = 2 * qc + 2
            ov0 = ps_ov.tile([65, QT], F32, tag="ov", name=f"ov0_{pair}{qc}")
            ov1 = ps_ov.tile([65, QT], F32, tag="ov", name=f"ov1_{pair}{qc}")
            pend = None
            for g in range(ngrp):
                sts = emit_scores(pair, qc, g)
                if pend is not None:
                    pg, p0, p1 = pend
                    emit_tail(pair, qc, pg, p0, p1, ov0, ov1, last=False)
                pend = (g, sts[0], sts[1])
                if g >= 1:
                    for _ in range(min(2, len(filler))):
                        filler.popleft()()
            pg, p0, p1 = pend
            emit_tail(pair, qc, pg, p0, p1, ov0, ov1, last=True)
            den, ous = emit_evac(pair, qc, ov0, ov1)
            rbs = emit_recip(pair, qc, den)
            emit_div(pair, qc, ous, rbs)

        for u in proj_units(0):
            u()
        for pair, qc in blocks:
            if qc in (0, 2):
                # the next pair's projections (esp. rope) must land well
                # before its qc=0 block; force-drain leftovers early
                while filler:
                    filler.popleft()()
            run_block(pair, qc)
            if qc == 0 and pair + 1 < NJT:
                filler.extend(proj_units(pair + 1))
            if pair == NJT - 1:
                # this qc's output projection is now unblocked; drip it in
                for mt in range(NMT):
                    filler.append(lambda qc=qc, mt=mt: outproj_unit(qc, mt))

        while filler:
            filler.popleft()()

    nc.compile()
    return nc


_NC_CACHE = {}


def _get_nc():
    if "nc" not in _NC_CACHE:
        _NC_CACHE["nc"] = _build_nc()
    return _NC_CACHE["nc"]


def _host_prep(x, wq, wk, wv, wo, token_positions):
    head_perm = np.concatenate([np.arange(0, DK, 2), np.arange(1, DK, 2)])
    pos = np.asarray(token_positions).astype(np.float32)
    half = np.arange(0, DK, 2, dtype=np.float32) / DK
    inv_freq = THETA ** (-half)
    ang = pos[:, None] * inv_freq[None, :]        # [S, 32]
    cosT = np.cos(ang).T.astype(np.float32)       # [32, S]
    sinT = np.sin(ang).T.astype(np.float32)
    c128 = np.tile(cosT, (4, 1)).astype(bf16)     # [128, S]
    s128 = np.concatenate([-sinT, sinT, -sinT, sinT], 0).astype(bf16)

    kp = np.arange(128)[:, None, None]
    jj = np.arange(4)[None, :, None]
    qf = np.arange(QT)[None, None, :]
    maskd = (qf >= kp + 128 * jj).astype(bf16)    # [128, 4, QT]

    def prep_qk(w, g):
        rows = w.reshape(H, DK, D)[g * HG:(g + 1) * HG][:, head_perm]
        return np.ascontiguousarray(rows.reshape(HG * DK, D).T)

    def prep_v(w, g):
        rows = w.reshape(H, DK, D)[g * HG:(g + 1) * HG]
        return np.ascontiguousarray(rows.reshape(HG * DK, D).T)

    common = {"c128": c128, "s128": s128, "maskd": maskd}
    in_maps = []
    for c in range(NCORES):
        b, g = c // NG, c % NG
        m = dict(common)
        xT = np.ascontiguousarray(x[b].T)
        m["xTb"] = xT[:, :QT].astype(bf16)
        m["x8T"] = xT.astype(f8e4)
        wqp, wkp, wvp = prep_qk(wq, g), prep_qk(wk, g), prep_v(wv, g)
        m["wqT"] = wqp.astype(bf16)
        m["wkT"] = wkp.astype(bf16)
        m["wq8T"] = wqp.astype(f8e4)
        m["wk8T"] = wkp.astype(f8e4)
        m["wvT"] = wvp.astype(bf16)
        m["wv8T"] = wvp.astype(f8e4)
        m["woT"] = np.ascontiguousarray(wo[:, g * HG * DK:(g + 1) * HG * DK].T
                                        ).astype(bf16)
        in_maps.append(m)
    return in_maps


def kernel(x, wq, wk, wv, wo, token_positions, _trace=False):
    x = np.asarray(x, dtype=np.float32)
    in_maps = _host_prep(x, wq, wk, wv, wo, token_positions)
    nc = _get_nc()
    res = run_bass_kernel_spmd(nc, in_maps, core_ids=list(range(NCORES)),
                               trace=_trace)
    out = np.zeros((B, S, D), np.float32)
    for b in range(B):
        acc = res.results[2 * b]["outT"].astype(np.float32) + \
            res.results[2 * b + 1]["outT"].astype(np.float32)
        out[b] = acc.T
    if _trace:
        kernel.last_results = res
    return out


# revision 21
# speedup vs baseline: 1.0255x; 1.0255x over previous
"""Trainium2 Bass kernel: causal multi-head self-attention with RoPE.

Problem: B=4, S=2048, D=1024, H=16, DK=64.  out = softmax(causal(qk^T/8)) v @ wo^T
with q,k RoPE-rotated.

Sharding: 8 cores = (batch b in 0..3) x (head-group g in 0..1, 8 heads each).
Each core computes its batch's QKV for its 8 heads, causal attention, and a
partial output projection; the host sums the two group-partials per batch.

Precision plan (validated on host): the softmax averages ~0.78*r keys at query
row r, so fp8e4m3 noise in E/V/Q/K washes out for r >= 512 but not below.
Everything touching query rows < 512 (tn=0 projections, qc=0 attention) and
keys < 512 for the qc=0 block stays bf16; the value path for qc>=1 runs fp8
with perf_mode=DoubleRow (2 k-subtiles per matmul).  The output projection
stays bf16 (fp8 relative error there hits the final result directly).

PE model (measured): matmul wall time ~ output free size x ~0.73-1.23ns
(drain-bound, clock depends on the HAM state); DoubleRow wins only by folding
2 contraction chunks into one output drain.  Scores are emitted as 64x64
quadrant-tiled matmuls (tile_position row+col groups) so up to 4 drain
concurrently, and the diagonal band skips fully-masked column prefixes
(stale PSUM there is bounded and the causal mask zeroes it; the st pool is
memset once at start).

Schedule: flattened (pair, qc) blocks with each pair's small qc=0 block
interleaved into the previous pair's big attention stream, erasing the
pair-boundary dips; next-pair q/k projections and the last pair's output
projections drip into the attention stream as filler.  Output partials are
written bf16 and summed on host.
"""
import os
import sys

for _p in ("/opt/trn_rl_repo", "/root/.axon_site/_ro/trn_rl_repo"):
    if os.path.isdir(_p) and _p not in sys.path:
        sys.path.insert(0, _p)

import numpy as np
import ml_dtypes

import concourse.bass as bass
import concourse.mybir as mybir
import concourse.tile as tile
from concourse import bacc
from concourse.bass_utils import run_bass_kernel_spmd

B, S, D, H = 4, 2048, 1024, 16
DK = D // H          # 64
HG = 8               # heads per group
NG = 2               # head groups (cores per batch)
THETA = 10000.0
NCORES = 8

BF16 = mybir.dt.bfloat16
F8 = mybir.dt.float8e4
F32 = mybir.dt.float32
bf16 = ml_dtypes.bfloat16
f8e4 = ml_dtypes.float8_e4m3

QT = 512             # q tile width (free dim)
NQT = S // QT        # 4
NKT = S // 128       # 16 k chunks
NJT = HG * DK // 128  # 4 j-tiles (head pairs)
NDC = D // 128       # 8 d chunks
NMT = D // 128       # 8 output m tiles

DR = mybir.MatmulPerfMode.DoubleRow


def _build_nc():
    nc = bacc.Bacc("TRN2", target_bir_lowering=False, debug=False)
    xTb = nc.dram_tensor("xTb", [D, QT], BF16, kind="ExternalInput").ap()
    x8T = nc.dram_tensor("x8T", [D, S], F8, kind="ExternalInput").ap()
    wqT = nc.dram_tensor("wqT", [D, HG * DK], BF16, kind="ExternalInput").ap()
    wkT = nc.dram_tensor("wkT", [D, HG * DK], BF16, kind="ExternalInput").ap()
    wq8T = nc.dram_tensor("wq8T", [D, HG * DK], F8, kind="ExternalInput").ap()
    wk8T = nc.dram_tensor("wk8T", [D, HG * DK], F8, kind="ExternalInput").ap()
    wvT = nc.dram_tensor("wvT", [D, HG * DK], BF16, kind="ExternalInput").ap()
    wv8T = nc.dram_tensor("wv8T", [D, HG * DK], F8, kind="ExternalInput").ap()
    woT = nc.dram_tensor("woT", [HG * DK, D], BF16, kind="ExternalInput").ap()
    c128 = nc.dram_tensor("c128", [128, S], BF16, kind="ExternalInput").ap()
    s128 = nc.dram_tensor("s128", [128, S], BF16, kind="ExternalInput").ap()
    maskd = nc.dram_tensor("maskd", [128, 4, QT], BF16, kind="ExternalInput").ap()
    outT = nc.dram_tensor("outT", [D, S], BF16, kind="ExternalOutput").ap()

    from contextlib import ExitStack
    with tile.TileContext(nc) as tc, ExitStack() as stk:
        pp = stk.enter_context(tc.tile_pool(name="persist", bufs=1))
        ep = stk.enter_context(tc.tile_pool(name="epool", bufs=6))
        sp = stk.enter_context(tc.tile_pool(name="smalls", bufs=2))
        qw = stk.enter_context(tc.tile_pool(name="qkvwork", bufs=2))
        ps_st = stk.enter_context(
            tc.tile_pool(name="ps_st", bufs=2, space="PSUM"))
        ps_ov = stk.enter_context(
            tc.tile_pool(name="ps_ov", bufs=2, space="PSUM"))
        ps_qkv = stk.enter_context(
            tc.tile_pool(name="ps_qkv", bufs=2, space="PSUM"))

        # ---------------- persistent tiles ----------------
        wo_sb = pp.tile([128, NJT, D], BF16)
        m_sb = pp.tile([128, 4, QT], BF16)
        qrot = pp.tile([128, NJT, S], BF16)
        krot = pp.tile([128, NJT, S], BF16)
        v_aug = pp.tile([128, 4, HG, 66], BF16)     # bf16 kc<4 (qc=0 path)
        v8 = pp.tile([128, NKT, HG, 66], F8)        # fp8 all kc (qc>=1 path)
        a_t = pp.tile([128, NJT, S], BF16)
        xb_sb = pp.tile([128, NDC, QT], BF16)       # x cols 0..511 (bf16 path)
        x8_sb = pp.tile([128, NDC, S - QT], F8)     # x cols 512.. (fp8 paths)
        wq_sb = pp.tile([128, NDC, HG * DK], BF16)
        wk_sb = pp.tile([128, NDC, HG * DK], BF16)
        wq8_sb = pp.tile([128, NDC, HG * DK], F8)
        wk8_sb = pp.tile([128, NDC, HG * DK], F8)
        c_sb = pp.tile([128, S], BF16)
        s_sb = pp.tile([128, S], BF16)

        nc.gpsimd.memset(v_aug[:, :, :, 64:65], 1.0)
        nc.gpsimd.memset(v8[:, :, :, 64:65], 1.0)
        # the diagonal-band score matmuls skip fully-masked column prefixes,
        # so exp() can read stale PSUM there (the mask zeroes it afterwards);
        # zero the two st pool buffers once so "stale" is always bounded.
        for i in range(2):
            sti = ps_st.tile([128, 2 * QT], F32, tag="st", name=f"stini{i}")
            nc.vector.memset(sti[:], 0.0)

        # ---------------- v projection (all heads) ----------------
        # bf16 for the first 4 k-chunks (they feed the bf16 qc=0 path),
        # fp8 DoubleRow for the rest; everything lands in v8, the bf16
        # chunks additionally in v_aug.
        with tc.tile_pool(name="wvtmp", bufs=1) as wvp:
            wv_sb = wvp.tile([128, NDC, HG * DK], BF16)
            wv8_sb = wvp.tile([128, NDC, HG * DK], F8)
            # minimal prologue: the first bf16 v-proj matmul needs only
            # wv + xb; everything else streams in behind it on 2 queues.
            for dc in range(NDC):
                nc.sync.dma_start(wv_sb[:, dc, :],
                                  wvT[dc * 128:(dc + 1) * 128, :])
                nc.sync.dma_start(xb_sb[:, dc, :],
                                  xTb[dc * 128:(dc + 1) * 128, :])
            nc.gpsimd.dma_start(
                x8_sb[:],
                x8T[:, QT:].rearrange("(dc p) c -> p dc c", p=128))
            nc.gpsimd.dma_start(
                wv8_sb[:],
                wv8T[:].rearrange("(dc p) c -> p dc c", p=128))
            nc.sync.dma_start(
                wq_sb[:], wqT[:].rearrange("(dc p) c -> p dc c", p=128))
            nc.sync.dma_start(
                wk_sb[:], wkT[:].rearrange("(dc p) c -> p dc c", p=128))
            nc.gpsimd.dma_start(
                wq8_sb[:], wq8T[:].rearrange("(dc p) c -> p dc c", p=128))
            nc.gpsimd.dma_start(
                wk8_sb[:], wk8T[:].rearrange("(dc p) c -> p dc c", p=128))
            nc.sync.dma_start(
                wo_sb[:], woT[:].rearrange("(jc p) c -> p jc c", p=128))
            nc.sync.dma_start(c_sb[:], c128[:])
            nc.sync.dma_start(s_sb[:], s128[:])
            nc.sync.dma_start(m_sb[:], maskd[:])
            for tt in range(NKT):
                ps = ps_qkv.tile([128, QT], F32, tag="qv")
                if tt < 4:
                    for dc in range(NDC):
                        nc.tensor.matmul(
                            ps[:],
                            xb_sb[:, dc, tt * 128:(tt + 1) * 128],
                            wv_sb[:, dc, :],
                            start=(dc == 0), stop=(dc == NDC - 1))
                    nc.vector.tensor_copy(
                        v_aug[:, tt, :, 0:64],
                        ps[:].rearrange("p (h d) -> p h d", h=HG))
                else:
                    for dc2 in range(NDC // 2):
                        nc.tensor.matmul(
                            ps[:],
                            x8_sb[:, 2 * dc2:2 * dc2 + 2,
                                  tt * 128 - QT:(tt + 1) * 128 - QT],
                            wv8_sb[:, 2 * dc2:2 * dc2 + 2, :],
                            start=(dc2 == 0), stop=(dc2 == NDC // 2 - 1),
                            perf_mode=DR)
                nc.vector.tensor_copy(
                    v8[:, tt, :, 0:64],
                    ps[:].rearrange("p (h d) -> p h d", h=HG))

        # ------------- projections + interleaved attention ---------
        def proj_unit(pair, name, w_sb, w8_sb, pre, tn):
            ps = ps_qkv.tile([128, QT], F32, tag="qv",
                             name=f"ps{name}{pair}{tn}")
            if tn == 0:
                for dc in range(NDC):
                    nc.tensor.matmul(
                        ps[:],
                        w_sb[:, dc, pair * 128:(pair + 1) * 128],
                        xb_sb[:, dc, :],
                        start=(dc == 0), stop=(dc == NDC - 1))
            else:
                for dc2 in range(NDC // 2):
                    nc.tensor.matmul(
                        ps[:],
                        w8_sb[:, 2 * dc2:2 * dc2 + 2,
                              pair * 128:(pair + 1) * 128],
                        x8_sb[:, 2 * dc2:2 * dc2 + 2,
                              (tn - 1) * QT:tn * QT],
                        start=(dc2 == 0), stop=(dc2 == NDC // 2 - 1),
                        perf_mode=DR)
            nc.vector.tensor_copy(pre[:, tn * QT:(tn + 1) * QT], ps[:])

        def rope_unit(pair, name, pre, dst):
            swp = qw.tile([128, S], BF16, tag="swp", name=f"swp{name}{pair}")
            for a in range(4):
                lo, sw = 32 * a, 32 * (a ^ 1)
                nc.sync.dma_start(swp[lo:lo + 32, :], pre[sw:sw + 32, :])
            nc.vector.tensor_mul(dst[:, pair, :], pre[:], c_sb[:])
            nc.vector.tensor_mul(swp[:], swp[:], s_sb[:])
            nc.vector.tensor_add(dst[:, pair, :], dst[:, pair, :], swp[:])

        def emit_scores(pair, qc, g):
            st0 = ps_st.tile([128, 2 * QT], F32, tag="st",
                             name=f"st0_{pair}{qc}{g}")
            st1 = ps_st.tile([128, 2 * QT], F32, tag="st",
                             name=f"st1_{pair}{qc}{g}")
            diag = g >= 2 * qc
            for half in range(2):
                kc = 2 * g + half
                for h01, st in ((0, st0), (1, st1)):
                    for kh in range(2):
                        qlo = 0
                        if diag:
                            jloc = kc - 4 * qc   # 0..3 within the band
                            qlo = 128 * jloc + 64 * kh
                        nc.tensor.matmul(
                            st[64 * kh:64 * kh + 64,
                               half * QT + qlo:(half + 1) * QT],
                            krot[64 * h01:64 * h01 + 64, pair,
                                 kc * 128 + 64 * kh:kc * 128 + 64 * kh + 64],
                            qrot[64 * h01:64 * h01 + 64, pair,
                                 qc * QT + qlo:(qc + 1) * QT],
                            start=True, stop=True,
                            tile_position=(64 * h01, 64 * kh))
            return st0, st1

        def emit_tail(pair, qc, g, st0, st1, ov0, ov1, last):
            diag = g >= 2 * qc
            for h01, st, ov in ((0, st0, ov0), (1, st1, ov1)):
                # exp lands in bf16 whenever a mask multiply follows (the
                # 16-bit DVE path is 2x the fp8 one); the mask multiply then
                # converts to fp8 on the way out for the DoubleRow matmul.
                edt = BF16 if (qc == 0 or diag) else F8
                e = ep.tile([128, 2 * QT], edt, tag="e", bufs=5,
                            name=f"e{pair}{qc}{g}{h01}")
                nc.scalar.activation(
                    e[:], st[:], mybir.ActivationFunctionType.Exp,
                    scale=0.125)
                if diag:
                    par = g - 2 * qc
                    if qc == 0:
                        e3 = e[:].rearrange("p (a q) -> p a q", a=2)
                        nc.vector.tensor_mul(
                            e3, e3, m_sb[:, 2 * par:2 * par + 2, :])
                    else:
                        e8 = ep.tile([128, 2 * QT], F8, tag="e8",
                                     bufs=3, name=f"e8{pair}{qc}{g}{h01}")
                        nc.vector.tensor_mul(
                            e8[:].rearrange("p (a q) -> p a q", a=2),
                            e[:].rearrange("p (a q) -> p a q", a=2),
                            m_sb[:, 2 * par:2 * par + 2, :])
                        e = e8
                if qc == 0:
                    for half in range(2):
                        kc = 2 * g + half
                        nc.tensor.matmul(
                            ov[:],
                            v_aug[:, kc, 2 * pair + h01, 0:65],
                            e[:, half * QT:(half + 1) * QT],
                            start=(kc == 0),
                            stop=(last and half == 1))
                else:
                    nc.tensor.matmul(
                        ov[:],
                        v8[:, 2 * g:2 * g + 2, 2 * pair + h01, 0:65],
                        e[:].rearrange("p (two q) -> p two q", two=2),
                        start=(g == 0),
                        stop=last,
                        perf_mode=DR)

        def emit_evac(pair, qc, ov0, ov1):
            """Part A: free the ov PSUM banks and stage the denominators."""
            den = sp.tile([2, QT], F32, tag="den", bufs=2,
                          name=f"den{pair}{qc}")
            ous = []
            for h01, ov in ((0, ov0), (1, ov1)):
                ou = ep.tile([65, QT], BF16, tag="ou", bufs=5,
                             name=f"ou{pair}{qc}{h01}")
                nc.vector.tensor_copy(ou[:], ov[:])
                nc.gpsimd.dma_start(den[h01:h01 + 1, :], ou[64:65, :])
                ous.append(ou)
            return den, ous

        def emit_recip(pair, qc, den):
            """Part B1: reciprocal + broadcast DMAs (deferred one qc)."""
            recip = sp.tile([2, QT], F32, tag="recip", name=f"rcp{pair}{qc}")
            nc.vector.reciprocal_approx_fast(recip[:], den[:])
            rbs = []
            for h01 in range(2):
                rb = sp.tile([64, QT], BF16, tag="rb", bufs=4,
                             name=f"rb{pair}{qc}{h01}")
                nc.gpsimd.dma_start(
                    rb[:],
                    recip[h01:h01 + 1, :]
                    .unsqueeze(1).to_broadcast((1, 64, QT)))
                rbs.append(rb)
            return rbs

        def emit_div(pair, qc, ous, rbs):
            """Part B2: the normalize multiplies (deferred further)."""
            nc.vector.tensor_mul(
                a_t[0:64, pair, qc * QT:(qc + 1) * QT],
                ous[0][0:64, :], rbs[0][:])
            an = sp.tile([64, QT], BF16, tag="an", bufs=3,
                         name=f"an{pair}{qc}")
            nc.vector.tensor_mul(an[:], ous[1][0:64, :], rbs[1][:])
            nc.sync.dma_start(
                a_t[64:128, pair, qc * QT:(qc + 1) * QT], an[:])

        def outproj_unit(qc, mt):
            op = ps_qkv.tile([128, QT], F32, tag="qv", name=f"op{qc}{mt}")
            for jc in range(NJT):
                nc.tensor.matmul(
                    op[:],
                    wo_sb[:, jc, mt * 128:(mt + 1) * 128],
                    a_t[:, jc, qc * QT:(qc + 1) * QT],
                    start=(jc == 0), stop=(jc == NJT - 1))
            ot = sp.tile([128, QT], BF16, tag="ot", bufs=3,
                         name=f"ot{qc}{mt}")
            nc.vector.tensor_copy(ot[:], op[:])
            nc.sync.dma_start(
                outT[mt * 128:(mt + 1) * 128, qc * QT:(qc + 1) * QT],
                ot[:])

        def proj_units(pair):
            preq = qw.tile([128, S], BF16, tag="preq", name=f"preq{pair}")
            prek = qw.tile([128, S], BF16, tag="prek", name=f"prek{pair}")
            for tn in range(NQT):
                yield lambda tn=tn: proj_unit(pair, "q", wq_sb, wq8_sb,
                                              preq, tn)
            yield lambda: rope_unit(pair, "q", preq, qrot)
            for tn in range(NQT):
                yield lambda tn=tn: proj_unit(pair, "k", wk_sb, wk8_sb,
                                              prek, tn)
            yield lambda: rope_unit(pair, "k", prek, krot)

        from collections import deque
        filler = deque()

        # per qc: how many filler units to drip in after each group
        # (placed mid-stream so the scores pipeline stays primed)
        UNIT_BUDGET = {0: 1, 1: 2, 2: 3, 3: 4}
        UNIT_BUDGET_LAST = {0: 2, 1: 6, 2: 10, 3: 14}

        def run_block(pair, qc, budget, npop):
            ngrp = 2 * qc + 2
            ov0 = ps_ov.tile([65, QT], F32, tag="ov", name=f"ov0_{pair}{qc}")
            ov1 = ps_ov.tile([65, QT], F32, tag="ov", name=f"ov1_{pair}{qc}")
            pend = None
            for g in range(ngrp):
                sts = emit_scores(pair, qc, g)
                if pend is not None:
                    pg, p0, p1 = pend
                    emit_tail(pair, qc, pg, p0, p1, ov0, ov1, last=False)
                pend = (g, sts[0], sts[1])
                if g >= 1 and budget > 0 and filler:
                    n = min(npop, budget, len(filler))
                    for _ in range(n):
                        filler.popleft()()
                    budget -= n
            pg, p0, p1 = pend
            emit_tail(pair, qc, pg, p0, p1, ov0, ov1, last=True)
            den, ous = emit_evac(pair, qc, ov0, ov1)
            rbs = emit_recip(pair, qc, den)
            emit_div(pair, qc, ous, rbs)

        for u in proj_units(0):
            u()
        for pair in range(NJT):
            last_pair = pair == NJT - 1
            if not last_pair:
                filler.extend(proj_units(pair + 1))
            budgets = UNIT_BUDGET_LAST if last_pair else UNIT_BUDGET
            for qc in range(NQT):
                run_block(pair, qc, budgets[qc], 2 if last_pair else 1)
                if last_pair:
                    for mt in range(NMT):
                        filler.append(
                            lambda qc=qc, mt=mt: outproj_unit(qc, mt))
            if not last_pair:
                while filler:
                    filler.popleft()()

        while filler:
            filler.popleft()()

    nc.compile()
    return nc


_NC_CACHE = {}


def _get_nc():
    if "nc" not in _NC_CACHE:
        _NC_CACHE["nc"] = _build_nc()
    return _NC_CACHE["nc"]


def _host_prep(x, wq, wk, wv, wo, token_positions):
    head_perm = np.concatenate([np.arange(0, DK, 2), np.arange(1, DK, 2)])
    pos = np.asarray(token_positions).astype(np.float32)
    half = np.arange(0, DK, 2, dtype=np.float32) / DK
    inv_freq = THETA ** (-half)
    ang = pos[:, None] * inv_freq[None, :]        # [S, 32]
    cosT = np.cos(ang).T.astype(np.float32)       # [32, S]
    sinT = np.sin(ang).T.astype(np.float32)
    c128 = np.tile(cosT, (4, 1)).astype(bf16)     # [128, S]
    s128 = np.concatenate([-sinT, sinT, -sinT, sinT], 0).astype(bf16)

    kp = np.arange(128)[:, None, None]
    jj = np.arange(4)[None, :, None]
    qf = np.arange(QT)[None, None, :]
    maskd = (qf >= kp + 128 * jj).astype(bf16)    # [128, 4, QT]

    def prep_qk(w, g):
        rows = w.reshape(H, DK, D)[g * HG:(g + 1) * HG][:, head_perm]
        return np.ascontiguousarray(rows.reshape(HG * DK, D).T)

    def prep_v(w, g):
        rows = w.reshape(H, DK, D)[g * HG:(g + 1) * HG]
        return np.ascontiguousarray(rows.reshape(HG * DK, D).T)

    common = {"c128": c128, "s128": s128, "maskd": maskd}
    in_maps = []
    for c in range(NCORES):
        b, g = c // NG, c % NG
        m = dict(common)
        xT = np.ascontiguousarray(x[b].T)
        m["xTb"] = xT[:, :QT].astype(bf16)
        m["x8T"] = xT.astype(f8e4)
        wqp, wkp, wvp = prep_qk(wq, g), prep_qk(wk, g), prep_v(wv, g)
        m["wqT"] = wqp.astype(bf16)
        m["wkT"] = wkp.astype(bf16)
        m["wq8T"] = wqp.astype(f8e4)
        m["wk8T"] = wkp.astype(f8e4)
        m["wvT"] = wvp.astype(bf16)
        m["wv8T"] = wvp.astype(f8e4)
        m["woT"] = np.ascontiguousarray(wo[:, g * HG * DK:(g + 1) * HG * DK].T
                                        ).astype(bf16)
        in_maps.append(m)
    return in_maps


def kernel(x, wq, wk, wv, wo, token_positions, _trace=False):
    x = np.asarray(x, dtype=np.float32)
    in_maps = _host_prep(x, wq, wk, wv, wo, token_positions)
    nc = _get_nc()
    res = run_bass_kernel_spmd(nc, in_maps, core_ids=list(range(NCORES)),
                               trace=_trace)
    out = np.zeros((B, S, D), np.float32)
    for b in range(B):
        acc = res.results[2 * b]["outT"].astype(np.float32) + \
            res.results[2 * b + 1]["outT"].astype(np.float32)
        out[b] = acc.T
    if _trace:
        kernel.last_results = res
    return out


# revision 22
# speedup vs baseline: 1.0811x; 1.0542x over previous
"""Trainium2 Bass kernel: causal multi-head self-attention with RoPE.

Problem: B=4, S=2048, D=1024, H=16, DK=64.  out = softmax(causal(qk^T/8)) v @ wo^T
with q,k RoPE-rotated.

Sharding: 8 cores = (batch b in 0..3) x (head-group g in 0..1, 8 heads each).
Each core computes its batch's QKV for its 8 heads, causal attention, and a
partial output projection; the host sums the two group-partials per batch.

Precision plan (validated on host): the softmax averages ~0.78*r keys at query
row r, so fp8e4m3 noise in E/V/Q/K washes out for r >= 512 but not below.
Everything touching query rows < 512 (tn=0 projections, qc=0 attention) and
keys < 512 for the qc=0 block stays bf16; the value path for qc>=1 runs fp8
with perf_mode=DoubleRow (2 k-subtiles per matmul).  The output projection
stays bf16 (fp8 relative error there hits the final result directly).

PE model (measured): matmul wall time ~ output free size x ~0.73-1.23ns
(drain-bound, clock depends on the HAM state); DoubleRow wins only by folding
2 contraction chunks into one output drain.  Scores are emitted as 64x64
quadrant-tiled matmuls (tile_position row+col groups) so up to 4 drain
concurrently, and the diagonal band skips fully-masked column prefixes
(stale PSUM there is bounded and the causal mask zeroes it; the st pool is
memset once at start).

Schedule: flattened (pair, qc) blocks with each pair's small qc=0 block
interleaved into the previous pair's big attention stream, erasing the
pair-boundary dips; next-pair q/k projections and the last pair's output
projections drip into the attention stream as filler.  Output partials are
written bf16 and summed on host.
"""
import os
import sys

for _p in ("/opt/trn_rl_repo", "/root/.axon_site/_ro/trn_rl_repo"):
    if os.path.isdir(_p) and _p not in sys.path:
        sys.path.insert(0, _p)

import numpy as np
import ml_dtypes

import concourse.bass as bass
import concourse.mybir as mybir
import concourse.tile as tile
from concourse import bacc
from concourse.bass_utils import run_bass_kernel_spmd

B, S, D, H = 4, 2048, 1024, 16
DK = D // H          # 64
HG = 8               # heads per group
NG = 2               # head groups (cores per batch)
THETA = 10000.0
NCORES = 8

BF16 = mybir.dt.bfloat16
F8 = mybir.dt.float8e4
F32 = mybir.dt.float32
bf16 = ml_dtypes.bfloat16
f8e4 = ml_dtypes.float8_e4m3

QT = 512             # q tile width (free dim)
NQT = S // QT        # 4
NKT = S // 128       # 16 k chunks
NJT = HG * DK // 128  # 4 j-tiles (head pairs)
NDC = D // 128       # 8 d chunks
NMT = D // 128       # 8 output m tiles

DR = mybir.MatmulPerfMode.DoubleRow


def _build_nc():
    nc = bacc.Bacc("TRN2", target_bir_lowering=False, debug=False)
    xTb = nc.dram_tensor("xTb", [D, QT], BF16, kind="ExternalInput").ap()
    x8T = nc.dram_tensor("x8T", [D, S], F8, kind="ExternalInput").ap()
    wqT = nc.dram_tensor("wqT", [D, HG * DK], BF16, kind="ExternalInput").ap()
    wkT = nc.dram_tensor("wkT", [D, HG * DK], BF16, kind="ExternalInput").ap()
    wq8T = nc.dram_tensor("wq8T", [D, HG * DK], F8, kind="ExternalInput").ap()
    wk8T = nc.dram_tensor("wk8T", [D, HG * DK], F8, kind="ExternalInput").ap()
    wvT = nc.dram_tensor("wvT", [D, HG * DK], BF16, kind="ExternalInput").ap()
    wv8T = nc.dram_tensor("wv8T", [D, HG * DK], F8, kind="ExternalInput").ap()
    woT = nc.dram_tensor("woT", [HG * DK, D], BF16, kind="ExternalInput").ap()
    c128 = nc.dram_tensor("c128", [128, S], BF16, kind="ExternalInput").ap()
    s128 = nc.dram_tensor("s128", [128, S], BF16, kind="ExternalInput").ap()
    maskd = nc.dram_tensor("maskd", [128, 4, QT], BF16, kind="ExternalInput").ap()
    outT = nc.dram_tensor("outT", [D, S], BF16, kind="ExternalOutput").ap()

    from contextlib import ExitStack
    with tile.TileContext(nc) as tc, ExitStack() as stk:
        pp = stk.enter_context(tc.tile_pool(name="persist", bufs=1))
        ep = stk.enter_context(tc.tile_pool(name="epool", bufs=6))
        sp = stk.enter_context(tc.tile_pool(name="smalls", bufs=2))
        qw = stk.enter_context(tc.tile_pool(name="qkvwork", bufs=2))
        ps_st = stk.enter_context(
            tc.tile_pool(name="ps_st", bufs=2, space="PSUM"))
        ps_ov = stk.enter_context(
            tc.tile_pool(name="ps_ov", bufs=2, space="PSUM"))
        ps_qkv = stk.enter_context(
            tc.tile_pool(name="ps_qkv", bufs=2, space="PSUM"))

        # ---------------- persistent tiles ----------------
        wo_sb = pp.tile([128, NJT, D], BF16)
        m_sb = pp.tile([128, 4, QT], BF16)
        qrot = pp.tile([128, NJT, S], BF16)
        krot = pp.tile([128, NJT, S], BF16)
        v_aug = pp.tile([128, 4, HG, 66], BF16)     # bf16 kc<4 (qc=0 path)
        v8 = pp.tile([128, NKT, HG, 66], F8)        # fp8 all kc (qc>=1 path)
        a_t = pp.tile([128, NJT, S], BF16)
        xb_sb = pp.tile([128, NDC, QT], BF16)       # x cols 0..511 (bf16 path)
        x8_sb = pp.tile([128, NDC, S - QT], F8)     # x cols 512.. (fp8 paths)
        wq_sb = pp.tile([128, NDC, HG * DK], BF16)
        wk_sb = pp.tile([128, NDC, HG * DK], BF16)
        wq8_sb = pp.tile([128, NDC, HG * DK], F8)
        wk8_sb = pp.tile([128, NDC, HG * DK], F8)
        c_sb = pp.tile([128, S], BF16)
        s_sb = pp.tile([128, S], BF16)

        nc.gpsimd.memset(v_aug[:, :, :, 64:65], 1.0)
        nc.gpsimd.memset(v8[:, :, :, 64:65], 1.0)
        # ---------------- v projection (all heads) ----------------
        # bf16 for the first 4 k-chunks (they feed the bf16 qc=0 path),
        # fp8 DoubleRow for the rest; everything lands in v8, the bf16
        # chunks additionally in v_aug.
        with tc.tile_pool(name="wvtmp", bufs=1) as wvp:
            wv_sb = wvp.tile([128, NDC, HG * DK], BF16)
            wv8_sb = wvp.tile([128, NDC, HG * DK], F8)
            # minimal prologue: the first bf16 v-proj matmul needs only
            # wv + xb; everything else streams in behind it on 2 queues.
            for dc in range(NDC):
                nc.sync.dma_start(wv_sb[:, dc, :],
                                  wvT[dc * 128:(dc + 1) * 128, :])
                nc.sync.dma_start(xb_sb[:, dc, :],
                                  xTb[dc * 128:(dc + 1) * 128, :])
            nc.gpsimd.dma_start(
                x8_sb[:],
                x8T[:, QT:].rearrange("(dc p) c -> p dc c", p=128))
            nc.gpsimd.dma_start(
                wv8_sb[:],
                wv8T[:].rearrange("(dc p) c -> p dc c", p=128))
            nc.sync.dma_start(
                wq_sb[:], wqT[:].rearrange("(dc p) c -> p dc c", p=128))
            nc.sync.dma_start(
                wk_sb[:], wkT[:].rearrange("(dc p) c -> p dc c", p=128))
            nc.gpsimd.dma_start(
                wq8_sb[:], wq8T[:].rearrange("(dc p) c -> p dc c", p=128))
            nc.gpsimd.dma_start(
                wk8_sb[:], wk8T[:].rearrange("(dc p) c -> p dc c", p=128))
            nc.sync.dma_start(
                wo_sb[:], woT[:].rearrange("(jc p) c -> p jc c", p=128))
            nc.sync.dma_start(c_sb[:], c128[:])
            nc.sync.dma_start(s_sb[:], s128[:])
            nc.sync.dma_start(m_sb[:], maskd[:])
            for tt in range(NKT):
                ps = ps_qkv.tile([128, QT], F32, tag="qv")
                if tt < 4:
                    for dc in range(NDC):
                        nc.tensor.matmul(
                            ps[:],
                            xb_sb[:, dc, tt * 128:(tt + 1) * 128],
                            wv_sb[:, dc, :],
                            start=(dc == 0), stop=(dc == NDC - 1))
                    nc.vector.tensor_copy(
                        v_aug[:, tt, :, 0:64],
                        ps[:].rearrange("p (h d) -> p h d", h=HG))
                else:
                    for dc2 in range(NDC // 2):
                        nc.tensor.matmul(
                            ps[:],
                            x8_sb[:, 2 * dc2:2 * dc2 + 2,
                                  tt * 128 - QT:(tt + 1) * 128 - QT],
                            wv8_sb[:, 2 * dc2:2 * dc2 + 2, :],
                            start=(dc2 == 0), stop=(dc2 == NDC // 2 - 1),
                            perf_mode=DR)
                nc.vector.tensor_copy(
                    v8[:, tt, :, 0:64],
                    ps[:].rearrange("p (h d) -> p h d", h=HG))

        # ------------- projections + interleaved attention ---------
        def proj_unit(pair, name, w_sb, w8_sb, pre, tn):
            ps = ps_qkv.tile([128, QT], F32, tag="qv",
                             name=f"ps{name}{pair}{tn}")
            if tn == 0:
                for dc in range(NDC):
                    nc.tensor.matmul(
                        ps[:],
                        w_sb[:, dc, pair * 128:(pair + 1) * 128],
                        xb_sb[:, dc, :],
                        start=(dc == 0), stop=(dc == NDC - 1))
            else:
                for dc2 in range(NDC // 2):
                    nc.tensor.matmul(
                        ps[:],
                        w8_sb[:, 2 * dc2:2 * dc2 + 2,
                              pair * 128:(pair + 1) * 128],
                        x8_sb[:, 2 * dc2:2 * dc2 + 2,
                              (tn - 1) * QT:tn * QT],
                        start=(dc2 == 0), stop=(dc2 == NDC // 2 - 1),
                        perf_mode=DR)
            nc.vector.tensor_copy(pre[:, tn * QT:(tn + 1) * QT], ps[:])

        def rope_unit(pair, name, pre, dst):
            swp = qw.tile([128, S], BF16, tag="swp", name=f"swp{name}{pair}")
            for a in range(4):
                lo, sw = 32 * a, 32 * (a ^ 1)
                nc.sync.dma_start(swp[lo:lo + 32, :], pre[sw:sw + 32, :])
            nc.vector.tensor_mul(dst[:, pair, :], pre[:], c_sb[:])
            nc.vector.tensor_mul(swp[:], swp[:], s_sb[:])
            nc.vector.tensor_add(dst[:, pair, :], dst[:, pair, :], swp[:])

        def emit_scores(pair, qc, g):
            st0 = ps_st.tile([128, 2 * QT], F32, tag="st",
                             name=f"st0_{pair}{qc}{g}")
            st1 = ps_st.tile([128, 2 * QT], F32, tag="st",
                             name=f"st1_{pair}{qc}{g}")
            for half in range(2):
                kc = 2 * g + half
                for h01, st in ((0, st0), (1, st1)):
                    lo = 64 * h01
                    nc.tensor.matmul(
                        st[:, half * QT:(half + 1) * QT],
                        krot[lo:lo + 64, pair, kc * 128:(kc + 1) * 128],
                        qrot[lo:lo + 64, pair, qc * QT:(qc + 1) * QT],
                        start=True, stop=True,
                        tile_position=(lo, 0))
            return st0, st1

        def emit_tail(pair, qc, g, st0, st1, ov0, ov1, last):
            diag = g >= 2 * qc
            for h01, st, ov in ((0, st0, ov0), (1, st1, ov1)):
                # exp lands in bf16 whenever a mask multiply follows (the
                # 16-bit DVE path is 2x the fp8 one); the mask multiply then
                # converts to fp8 on the way out for the DoubleRow matmul.
                edt = BF16 if (qc == 0 or diag) else F8
                e = ep.tile([128, 2 * QT], edt, tag="e", bufs=5,
                            name=f"e{pair}{qc}{g}{h01}")
                nc.scalar.activation(
                    e[:], st[:], mybir.ActivationFunctionType.Exp,
                    scale=0.125)
                if diag:
                    par = g - 2 * qc
                    if qc == 0:
                        e3 = e[:].rearrange("p (a q) -> p a q", a=2)
                        nc.vector.tensor_mul(
                            e3, e3, m_sb[:, 2 * par:2 * par + 2, :])
                    else:
                        e8 = ep.tile([128, 2 * QT], F8, tag="e8",
                                     bufs=3, name=f"e8{pair}{qc}{g}{h01}")
                        nc.vector.tensor_mul(
                            e8[:].rearrange("p (a q) -> p a q", a=2),
                            e[:].rearrange("p (a q) -> p a q", a=2),
                            m_sb[:, 2 * par:2 * par + 2, :])
                        e = e8
                if qc == 0:
                    for half in range(2):
                        kc = 2 * g + half
                        nc.tensor.matmul(
                            ov[:],
                            v_aug[:, kc, 2 * pair + h01, 0:65],
                            e[:, half * QT:(half + 1) * QT],
                            start=(kc == 0),
                            stop=(last and half == 1))
                else:
                    nc.tensor.matmul(
                        ov[:],
                        v8[:, 2 * g:2 * g + 2, 2 * pair + h01, 0:65],
                        e[:].rearrange("p (two q) -> p two q", two=2),
                        start=(g == 0),
                        stop=last,
                        perf_mode=DR)

        def emit_evac(pair, qc, ov0, ov1):
            """Part A: free the ov PSUM banks and stage the denominators."""
            den = sp.tile([2, QT], F32, tag="den", bufs=2,
                          name=f"den{pair}{qc}")
            ous = []
            for h01, ov in ((0, ov0), (1, ov1)):
                ou = ep.tile([65, QT], BF16, tag="ou", bufs=5,
                             name=f"ou{pair}{qc}{h01}")
                nc.vector.tensor_copy(ou[:], ov[:])
                nc.gpsimd.dma_start(den[h01:h01 + 1, :], ou[64:65, :])
                ous.append(ou)
            return den, ous

        def emit_recip(pair, qc, den):
            """Part B1: reciprocal + broadcast DMAs (deferred one qc)."""
            recip = sp.tile([2, QT], F32, tag="recip", name=f"rcp{pair}{qc}")
            nc.vector.reciprocal_approx_fast(recip[:], den[:])
            rbs = []
            for h01 in range(2):
                rb = sp.tile([64, QT], BF16, tag="rb", bufs=4,
                             name=f"rb{pair}{qc}{h01}")
                nc.gpsimd.dma_start(
                    rb[:],
                    recip[h01:h01 + 1, :]
                    .unsqueeze(1).to_broadcast((1, 64, QT)))
                rbs.append(rb)
            return rbs

        def emit_div(pair, qc, ous, rbs):
            """Part B2: the normalize multiplies (deferred further)."""
            nc.vector.tensor_mul(
                a_t[0:64, pair, qc * QT:(qc + 1) * QT],
                ous[0][0:64, :], rbs[0][:])
            an = sp.tile([64, QT], BF16, tag="an", bufs=3,
                         name=f"an{pair}{qc}")
            nc.vector.tensor_mul(an[:], ous[1][0:64, :], rbs[1][:])
            nc.sync.dma_start(
                a_t[64:128, pair, qc * QT:(qc + 1) * QT], an[:])

        def outproj_unit(qc, mt):
            op = ps_qkv.tile([128, QT], F32, tag="qv", name=f"op{qc}{mt}")
            for jc in range(NJT):
                nc.tensor.matmul(
                    op[:],
                    wo_sb[:, jc, mt * 128:(mt + 1) * 128],
                    a_t[:, jc, qc * QT:(qc + 1) * QT],
                    start=(jc == 0), stop=(jc == NJT - 1))
            ot = sp.tile([128, QT], BF16, tag="ot", bufs=3,
                         name=f"ot{qc}{mt}")
            nc.vector.tensor_copy(ot[:], op[:])
            nc.sync.dma_start(
                outT[mt * 128:(mt + 1) * 128, qc * QT:(qc + 1) * QT],
                ot[:])

        def proj_units(pair):
            preq = qw.tile([128, S], BF16, tag="preq", name=f"preq{pair}")
            prek = qw.tile([128, S], BF16, tag="prek", name=f"prek{pair}")
            for tn in range(NQT):
                yield lambda tn=tn: proj_unit(pair, "q", wq_sb, wq8_sb,
                                              preq, tn)
            yield lambda: rope_unit(pair, "q", preq, qrot)
            for tn in range(NQT):
                yield lambda tn=tn: proj_unit(pair, "k", wk_sb, wk8_sb,
                                              prek, tn)
            yield lambda: rope_unit(pair, "k", prek, krot)

        from collections import deque
        filler = deque()

        # per qc: how many filler units to drip in after each group
        # (placed mid-stream so the scores pipeline stays primed)
        UNIT_BUDGET = {0: 1, 1: 2, 2: 3, 3: 4}
        UNIT_BUDGET_LAST = {0: 2, 1: 6, 2: 10, 3: 14}

        def run_block(pair, qc, budget, npop):
            ngrp = 2 * qc + 2
            ov0 = ps_ov.tile([65, QT], F32, tag="ov", name=f"ov0_{pair}{qc}")
            ov1 = ps_ov.tile([65, QT], F32, tag="ov", name=f"ov1_{pair}{qc}")
            pend = None
            for g in range(ngrp):
                sts = emit_scores(pair, qc, g)
                if pend is not None:
                    pg, p0, p1 = pend
                    emit_tail(pair, qc, pg, p0, p1, ov0, ov1, last=False)
                pend = (g, sts[0], sts[1])
                if g >= 1 and budget > 0 and filler:
                    n = min(npop, budget, len(filler))
                    for _ in range(n):
                        filler.popleft()()
                    budget -= n
            pg, p0, p1 = pend
            emit_tail(pair, qc, pg, p0, p1, ov0, ov1, last=True)
            den, ous = emit_evac(pair, qc, ov0, ov1)
            rbs = emit_recip(pair, qc, den)
            emit_div(pair, qc, ous, rbs)

        for u in proj_units(0):
            u()
        for pair in range(NJT):
            last_pair = pair == NJT - 1
            if not last_pair:
                filler.extend(proj_units(pair + 1))
            budgets = UNIT_BUDGET_LAST if last_pair else UNIT_BUDGET
            for qc in range(NQT):
                run_block(pair, qc, budgets[qc], 2 if last_pair else 1)
                if last_pair:
                    for mt in range(NMT):
                        filler.append(
                            lambda qc=qc, mt=mt: outproj_unit(qc, mt))
            if not last_pair:
                while filler:
                    filler.popleft()()

        while filler:
            filler.popleft()()

    nc.compile()
    return nc


_NC_CACHE = {}


def _get_nc():
    if "nc" not in _NC_CACHE:
        _NC_CACHE["nc"] = _build_nc()
    return _NC_CACHE["nc"]


def _host_prep(x, wq, wk, wv, wo, token_positions):
    head_perm = np.concatenate([np.arange(0, DK, 2), np.arange(1, DK, 2)])
    pos = np.asarray(token_positions).astype(np.float32)
    half = np.arange(0, DK, 2, dtype=np.float32) / DK
    inv_freq = THETA ** (-half)
    ang = pos[:, None] * inv_freq[None, :]        # [S, 32]
    cosT = np.cos(ang).T.astype(np.float32)       # [32, S]
    sinT = np.sin(ang).T.astype(np.float32)
    c128 = np.tile(cosT, (4, 1)).astype(bf16)     # [128, S]
    s128 = np.concatenate([-sinT, sinT, -sinT, sinT], 0).astype(bf16)

    kp = np.arange(128)[:, None, None]
    jj = np.arange(4)[None, :, None]
    qf = np.arange(QT)[None, None, :]
    maskd = (qf >= kp + 128 * jj).astype(bf16)    # [128, 4, QT]

    def prep_qk(w, g):
        rows = w.reshape(H, DK, D)[g * HG:(g + 1) * HG][:, head_perm]
        return np.ascontiguousarray(rows.reshape(HG * DK, D).T)

    def prep_v(w, g):
        rows = w.reshape(H, DK, D)[g * HG:(g + 1) * HG]
        return np.ascontiguousarray(rows.reshape(HG * DK, D).T)

    common = {"c128": c128, "s128": s128, "maskd": maskd}
    in_maps = []
    for c in range(NCORES):
        b, g = c // NG, c % NG
        m = dict(common)
        xT = np.ascontiguousarray(x[b].T)
        m["xTb"] = xT[:, :QT].astype(bf16)
        m["x8T"] = xT.astype(f8e4)
        wqp, wkp, wvp = prep_qk(wq, g), prep_qk(wk, g), prep_v(wv, g)
        m["wqT"] = wqp.astype(bf16)
        m["wkT"] = wkp.astype(bf16)
        m["wq8T"] = wqp.astype(f8e4)
        m["wk8T"] = wkp.astype(f8e4)
        m["wvT"] = wvp.astype(bf16)
        m["wv8T"] = wvp.astype(f8e4)
        m["woT"] = np.ascontiguousarray(wo[:, g * HG * DK:(g + 1) * HG * DK].T
                                        ).astype(bf16)
        in_maps.append(m)
    return in_maps


def kernel(x, wq, wk, wv, wo, token_positions, _trace=False):
    x = np.asarray(x, dtype=np.float32)
    in_maps = _host_prep(x, wq, wk, wv, wo, token_positions)
    nc = _get_nc()
    res = run_bass_kernel_spmd(nc, in_maps, core_ids=list(range(NCORES)),
                               trace=_trace)
    out = np.zeros((B, S, D), np.float32)
    for b in range(B):
        acc = res.results[2 * b]["outT"].astype(np.float32) + \
            res.results[2 * b + 1]["outT"].astype(np.float32)
        out[b] = acc.T
    if _trace:
        kernel.last_results = res
    return out
